# revision 14
# baseline (speedup 1.0000x reference)
"""Multi-head attention forward on 8 Trainium2 NeuronCores (Bass/Tile).

Problem: B=2, S=2048, HIDDEN=2048, HEADS=16, D_K=128, fp32 I/O,
mask all-ones (eval). torch-Linear convention: y = x @ W.T.

Sharding (head + output-row parallel, two AllToAlls, no all-reduce):
  - core c (0..7) owns heads {2c, 2c+1} for BOTH batches.
  - Phase A: per batch, project Q,K (transposed layout [d, s]) and V
    (natural layout [s, d]) for the core's 2 heads. bf16 inputs, fp32 acc.
  - Phase B: per (batch, q-block, head): scoresT tiles [k, q] via
    Kh^T-stationary matmuls, exp on ACT (1/sqrt(dk) folded into the
    activation scale), PV accumulation in the transposed layout (no
    attention transposes anywhere). Softmax denominators: DVE
    accumulation of exp tiles + one GPSIMD partition_all_reduce, so the
    PE never waits on softmax bookkeeping. Softmax without
    max-subtraction: scores are O(few) here, mathematically identical
    to the reference.
  - One AllToAll per batch (8 ranks, 1MB): fired as soon as that batch's
    attention finishes, so A2A#0 hides under batch-1 attention and A2A#1
    under batch-0's output projection.
  - Phase D: out_chunk = concat_chunk @ W_o.T per batch; each core
    produces 256 output rows per batch.
Host side: pre-transpose/cast inputs to bf16 (PE-tile-friendly layouts,
single big DMAs), slice weights per core, scatter-gather the per-core
[512, 2048] fp32 chunks into the full [B, S, HIDDEN] output.
"""

import math
from contextlib import ExitStack

import ml_dtypes
import numpy as np

import concourse.bass as bass
import concourse.bass_isa as bass_isa
import concourse.tile as tile
from concourse import bacc, mybir
from concourse.bass_utils import run_bass_kernel_spmd

BF16 = mybir.dt.bfloat16
F32 = mybir.dt.float32
NPBF16 = ml_dtypes.bfloat16

HIDDEN = 2048
HEADS = 16
D_K = 128
B = 2
N_CORES = 8
HPC = HEADS // N_CORES          # heads per core (2)
DPC = HPC * D_K                 # concat cols per core (256)
NHT = HIDDEN // 128             # 16 hidden-dim 128-tiles


def _mha_kernel(ctx: ExitStack, tc: tile.TileContext, aps: dict, S: int):
    nc = tc.nc
    NKT = S // 128                   # seq 128-tiles
    SBLK = min(512, S)               # matmul moving-dim block
    NSB = S // SBLK
    QBLK = SBLK
    NQB = NSB
    SCB = S // N_CORES               # per-batch output rows per core (256)
    OBLK = 512
    NOB = HIDDEN // OBLK
    scale = 1.0 / math.sqrt(D_K)

    qT, kT, vT = aps["qT"], aps["kT"], aps["vT"]   # per batch [HIDDEN, S]
    wqT, wkT, wvT = aps["wqT"], aps["wkT"], aps["wvT"]  # [128, NHT*DPC]
    woT = aps["woT"]                                # [128, NHT*HIDDEN]
    out = aps["out"]                                # [B*SCB, HIDDEN] f32
    a2a_in = aps["a2a_in"]                          # per batch [8*DPC, SCB]
    a2a_out = aps["a2a_out"]                        # per batch [8*DPC, SCB]

    # ---- resident weights (pre-tiled on host; one big DMA each) ----
    w_pool = ctx.enter_context(tc.tile_pool(name="wqkv", bufs=1))
    wq_sb = w_pool.tile([128, NHT * DPC], BF16, tag="wq")
    wk_sb = w_pool.tile([128, NHT * DPC], BF16, tag="wk")
    wv_sb = w_pool.tile([128, NHT * DPC], BF16, tag="wv")
    nc.sync.dma_start(out=wq_sb[:], in_=wqT[:, :])

    # ---- resident projection outputs (both batches) ----
    proj_pool = ctx.enter_context(tc.tile_pool(name="proj", bufs=1))
    qh_sb = [proj_pool.tile([128, HPC * S], BF16, tag=f"qh{b}", name=f"qh{b}")
             for b in range(B)]
    kh_sb = [proj_pool.tile([128, HPC * S], BF16, tag=f"kh{b}", name=f"kh{b}")
             for b in range(B)]
    vh_sb = [proj_pool.tile([128, NKT * DPC], BF16, tag=f"vh{b}", name=f"vh{b}")
             for b in range(B)]

    # ================= per-batch: A (proj) then B (attention) =========
    def phase_a(b):
        with tc.tile_pool(name="psA", bufs=8, space="PSUM") as psA, \
             tc.tile_pool(name="xrow", bufs=4) as xrow_pool:
            # qhT / khT: [d, s] layout.  psum[dt*NSB+sb] accumulates over ht.
            for wi, (src, w_sb, dst) in enumerate(
                    ((qT[b], wq_sb, qh_sb[b]), (kT[b], wk_sb, kh_sb[b]))):
                if b == 0 and wi == 0:
                    nc.sync.dma_start(out=wk_sb[:], in_=wkT[:, :])
                ps = [psA.tile([128, SBLK], F32, tag="psA", name=f"psA{i}")
                      for i in range(HPC * NSB)]
                for ht in range(NHT):
                    row = xrow_pool.tile([128, S], BF16, tag="xrow")
                    nc.sync.dma_start(out=row[:],
                                      in_=src[ht * 128:(ht + 1) * 128, :])
                    for dt in range(HPC):
                        for sb in range(NSB):
                            nc.tensor.matmul(
                                ps[dt * NSB + sb][:],
                                lhsT=w_sb[:, ht * DPC + dt * 128:
                                          ht * DPC + (dt + 1) * 128],
                                rhs=row[:, sb * SBLK:(sb + 1) * SBLK],
                                start=(ht == 0), stop=(ht == NHT - 1))
                for dt in range(HPC):
                    for sb in range(NSB):
                        nc.vector.tensor_copy(
                            dst[:, dt * S + sb * SBLK: dt * S + (sb + 1) * SBLK],
                            ps[dt * NSB + sb][:])

            # vh: natural [s, d] layout; stationary = vT tiles.
            if b == 0:
                nc.sync.dma_start(out=wv_sb[:], in_=wvT[:, :])
            with tc.tile_pool(name="vfull", bufs=4) as vfull:
                nhalf = (NKT + 7) // 8
                for half in range(nhalf):
                    sts = range(half * 8, min((half + 1) * 8, NKT))
                    ps = {st: psA.tile([128, DPC], F32, tag="psA",
                                       name=f"psV{st}")
                          for st in sts}
                    for ht in range(NHT):
                        vr = vfull.tile([128, S], BF16, tag="vrow")
                        nc.sync.dma_start(
                            out=vr[:], in_=vT[b][ht * 128:(ht + 1) * 128, :])
                        for st in sts:
                            nc.tensor.matmul(
                                ps[st][:],
                                lhsT=vr[:, st * 128:(st + 1) * 128],
                                rhs=wv_sb[:, ht * DPC:(ht + 1) * DPC],
                                start=(ht == 0), stop=(ht == NHT - 1))
                    for st in sts:
                        nc.vector.tensor_copy(
                            vh_sb[b][:, st * DPC:(st + 1) * DPC], ps[st][:])

    def phase_b(b):
        with tc.tile_pool(name="pss", bufs=5, space="PSUM") as pss_pool, \
             tc.tile_pool(name="pspv", bufs=3, space="PSUM") as pspv_pool, \
             tc.tile_pool(name="es", bufs=8) as es_pool, \
             tc.tile_pool(name="dtree", bufs=10) as dt_pool, \
             tc.tile_pool(name="acc", bufs=3) as acc_pool, \
             tc.tile_pool(name="rb", bufs=3) as rb_pool, \
             tc.tile_pool(name="ao", bufs=4) as ao_pool:
            for qb in range(NQB):
                for l in range(HPC):
                    pv = pspv_pool.tile([128, QBLK], F32, tag="pv")
                    # denominator tree: bf16 pair adds (2x DVE mode)
                    pending = {}
                    for kt in range(NKT):
                        lhs_k = kh_sb[b][:, l * S + kt * 128:
                                         l * S + (kt + 1) * 128]
                        lhs_v = vh_sb[b][:, kt * DPC + l * 128:
                                         kt * DPC + (l + 1) * 128]
                        ps = pss_pool.tile([128, QBLK], F32, tag="pss")
                        nc.tensor.matmul(
                            ps[:], lhsT=lhs_k,
                            rhs=qh_sb[b][:, l * S + qb * QBLK:
                                         l * S + (qb + 1) * QBLK],
                            start=True, stop=True)
                        es = es_pool.tile([128, QBLK], BF16, tag="es")
                        nc.scalar.activation(
                            es[:], ps[:], mybir.ActivationFunctionType.Exp,
                            scale=scale)
                        nc.tensor.matmul(
                            pv[:], lhsT=lhs_v, rhs=es[:],
                            start=(kt == 0), stop=(kt == NKT - 1))
                        # fold into the bf16 pair tree as tiles arrive
                        node, level = es, 0
                        while level in pending:
                            nxt = dt_pool.tile([128, QBLK], BF16, tag="dtree")
                            nc.vector.tensor_add(nxt[:], pending.pop(level)[:],
                                                 node[:])
                            node, level = nxt, level + 1
                        pending[level] = node
                    nodes = [pending[k] for k in sorted(pending)]
                    node = nodes[0]
                    for other in nodes[1:]:
                        nxt = dt_pool.tile([128, QBLK], BF16, tag="dtree")
                        nc.vector.tensor_add(nxt[:], node[:], other[:])
                        node = nxt
                    acc = acc_pool.tile([128, QBLK], F32, tag="acc")
                    nc.vector.tensor_copy(acc[:], node[:])
                    rb = rb_pool.tile([128, QBLK], F32, tag="rb")
                    nc.gpsimd.partition_all_reduce(
                        rb[:], acc[:], channels=128,
                        reduce_op=bass_isa.ReduceOp.add)
                    nc.vector.reciprocal_approx_fast(rb[:], rb[:])
                    ao = ao_pool.tile([128, QBLK], BF16, tag="ao")
                    nc.vector.tensor_mul(ao[:], pv[:], rb[:])
                    # scatter into a2a_in[b]: chunk m rows [m*DPC+l*128, +128)
                    q0 = qb * QBLK
                    while q0 < (qb + 1) * QBLK:
                        m = q0 // SCB
                        cend = min((qb + 1) * QBLK, (m + 1) * SCB)
                        nc.sync.dma_start(
                            out=a2a_in[b][m * DPC + l * 128:
                                          m * DPC + (l + 1) * 128,
                                          q0 - m * SCB: cend - m * SCB],
                            in_=ao[:, q0 - qb * QBLK: cend - qb * QBLK])
                        q0 = cend

    colls = []
    cc_tiles = []
    cc_pool = None
    wo_sb = None
    for b in range(B):
        phase_a(b)
        if b == 1:
            # W_o arrives during batch-1 attention (SBUF is free by then)
            wo_pool = ctx.enter_context(tc.tile_pool(name="wo", bufs=1))
            wo_sb = wo_pool.tile([128, NHT * HIDDEN], BF16, tag="wo")
            nc.sync.dma_start(out=wo_sb[:], in_=woT[:, :])
        phase_b(b)
        if cc_pool is None:
            cc_pool = ctx.enter_context(tc.tile_pool(name="cc", bufs=B))
        coll = nc.gpsimd.collective_compute(
            "AllToAll", mybir.AluOpType.bypass,
            replica_groups=[list(range(N_CORES))],
            ins=[a2a_in[b][:, :]], outs=[a2a_out[b][:, :]])
        colls.append(coll)
        cc_sb = cc_pool.tile([128, NHT * SCB], BF16, tag="cc", name=f"cc{b}")
        dma = nc.sync.dma_start(
            out=cc_sb[:].rearrange("p (t s) -> p t s", t=NHT),
            in_=a2a_out[b][:, :].rearrange("(t p) s -> p t s", p=128))
        tile.add_dep_helper(dma.ins, coll.ins,
                            reason="a2a_out after collective")
        cc_tiles.append(cc_sb)

    # ================= Phase D: output projection (per batch) =========
    with tc.tile_pool(name="pso", bufs=3, space="PSUM") as pso_pool, \
         tc.tile_pool(name="osb", bufs=4) as osb_pool:
        NST = (SCB + 127) // 128
        for b in range(B):
            cc_sb = cc_tiles[b]
            for st in range(NST):
                mrows = min(128, SCB - st * 128)
                pso = [pso_pool.tile([128, OBLK], F32, tag="pso",
                                     name=f"pso{i}") for i in range(NOB)]
                for ht in range(NHT):
                    lhs = cc_sb[:, ht * SCB + st * 128:
                                ht * SCB + st * 128 + mrows]
                    for ot in range(NOB):
                        nc.tensor.matmul(
                            pso[ot][:mrows, :], lhsT=lhs,
                            rhs=wo_sb[:, ht * HIDDEN + ot * OBLK:
                                      ht * HIDDEN + (ot + 1) * OBLK],
                            start=(ht == 0), stop=(ht == NHT - 1))
                for ot in range(NOB):
                    osb = osb_pool.tile([128, OBLK], F32, tag="osb")
                    nc.vector.tensor_copy(osb[:mrows, :], pso[ot][:mrows, :])
                    nc.sync.dma_start(
                        out=out[b * SCB + st * 128: b * SCB + st * 128 + mrows,
                                ot * OBLK:(ot + 1) * OBLK],
                        in_=osb[:mrows, :])


def build_nc(S: int):
    nc = bacc.Bacc("TRN2", target_bir_lowering=False, debug=False,
                   enable_asserts=False, num_devices=N_CORES)
    SCB = S // N_CORES
    aps = {
        "qT": [nc.dram_tensor(f"qT{b}", [HIDDEN, S], BF16,
                              kind="ExternalInput").ap() for b in range(B)],
        "kT": [nc.dram_tensor(f"kT{b}", [HIDDEN, S], BF16,
                              kind="ExternalInput").ap() for b in range(B)],
        "vT": [nc.dram_tensor(f"vT{b}", [HIDDEN, S], BF16,
                              kind="ExternalInput").ap() for b in range(B)],
        "wqT": nc.dram_tensor("wqT", [128, NHT * DPC], BF16,
                              kind="ExternalInput").ap(),
        "wkT": nc.dram_tensor("wkT", [128, NHT * DPC], BF16,
                              kind="ExternalInput").ap(),
        "wvT": nc.dram_tensor("wvT", [128, NHT * DPC], BF16,
                              kind="ExternalInput").ap(),
        "woT": nc.dram_tensor("woT", [128, NHT * HIDDEN], BF16,
                              kind="ExternalInput").ap(),
        "out": nc.dram_tensor("out", [B * SCB, HIDDEN], F32,
                              kind="ExternalOutput").ap(),
        "a2a_in": [nc.dram_tensor(f"a2a_in{b}", [N_CORES * DPC, SCB],
                                  BF16).ap() for b in range(B)],
        "a2a_out": [nc.dram_tensor(f"a2a_out{b}", [N_CORES * DPC, SCB],
                                   BF16).ap() for b in range(B)],
    }
    with tile.TileContext(nc) as tc:
        with ExitStack() as ctx:
            _mha_kernel(ctx, tc, aps, S)
    nc.compile()
    return nc


_NC_CACHE: dict = {}


def _tile_weight(w_slice_T):
    """[H, D] -> [128, (H//128)*D] with 128-row tiles laid out consecutively."""
    H, D = w_slice_T.shape
    return np.ascontiguousarray(
        w_slice_T.reshape(H // 128, 128, D).transpose(1, 0, 2).reshape(
            128, (H // 128) * D))


def make_in_maps(q, k, v, w_q, w_k, w_v, w_o):
    """Host-side shard/cast. Returns per-core input dicts."""
    qT = [np.ascontiguousarray(q[b].T).astype(NPBF16) for b in range(B)]
    kT = [np.ascontiguousarray(k[b].T).astype(NPBF16) for b in range(B)]
    vT = [np.ascontiguousarray(v[b].T).astype(NPBF16) for b in range(B)]
    woT = _tile_weight(np.ascontiguousarray(w_o.T).astype(NPBF16))
    in_maps = []
    for c in range(N_CORES):
        d0 = c * DPC
        m = {}
        for b in range(B):
            m[f"qT{b}"] = qT[b]
            m[f"kT{b}"] = kT[b]
            m[f"vT{b}"] = vT[b]
        m["wqT"] = _tile_weight(
            np.ascontiguousarray(w_q[d0:d0 + DPC, :].T).astype(NPBF16))
        m["wkT"] = _tile_weight(
            np.ascontiguousarray(w_k[d0:d0 + DPC, :].T).astype(NPBF16))
        m["wvT"] = _tile_weight(
            np.ascontiguousarray(w_v[d0:d0 + DPC, :].T).astype(NPBF16))
        m["woT"] = woT
        in_maps.append(m)
    return in_maps


def kernel(q, k, v, mask, w_q, w_k, w_v, w_o, _trace=False):
    q = np.asarray(q, np.float32)
    k = np.asarray(k, np.float32)
    v = np.asarray(v, np.float32)
    mask = np.asarray(mask)
    w_q = np.asarray(w_q, np.float32)
    w_k = np.asarray(w_k, np.float32)
    w_v = np.asarray(w_v, np.float32)
    w_o = np.asarray(w_o, np.float32)
    S = q.shape[1]

    if not np.all(mask != 0):
        # General-mask fallback (never hit for the eval problem: mask is all
        # ones).  Computed on host for correctness.
        return _numpy_reference(q, k, v, mask, w_q, w_k, w_v, w_o)

    if S not in _NC_CACHE:
        _NC_CACHE[S] = build_nc(S)
    nc = _NC_CACHE[S]

    in_maps = make_in_maps(q, k, v, w_q, w_k, w_v, w_o)
    res = run_bass_kernel_spmd(nc, in_maps, core_ids=list(range(N_CORES)),
                               trace=_trace)

    SCB = S // N_CORES
    out = np.empty((B, S, HIDDEN), np.float32)
    for c in range(N_CORES):
        for b in range(B):
            out[b, c * SCB:(c + 1) * SCB, :] = \
                res.results[c]["out"][b * SCB:(b + 1) * SCB, :]
    if _trace:
        return out, res
    return out


def _numpy_reference(q, k, v, mask, w_q, w_k, w_v, w_o):
    Bn, S, H = q.shape
    dk = H // HEADS

    def split_heads(x, w):
        y = x @ w.T
        return y.reshape(Bn, S, HEADS, dk).transpose(0, 2, 1, 3)

    qh = split_heads(q, w_q)
    kh = split_heads(k, w_k)
    vh = split_heads(v, w_v)
    s = np.einsum("bhqd,bhkd->bhqk", qh, kh) / np.sqrt(np.float32(dk))
    s = np.where(mask[:, None, :, :] == 0, np.float32(-1e9), s)
    s = s - s.max(-1, keepdims=True)
    e = np.exp(s)
    a = e / e.sum(-1, keepdims=True)
    o = np.einsum("bhqk,bhkd->bhqd", a, vh)
    o = o.transpose(0, 2, 1, 3).reshape(Bn, S, H)
    return (o @ w_o.T).astype(np.float32)


# revision 19
# speedup vs baseline: 1.0756x; 1.0756x over previous
"""Multi-head attention forward on 8 Trainium2 NeuronCores (Bass/Tile).

Problem: B=2, S=2048, HIDDEN=2048, HEADS=16, D_K=128, fp32 I/O,
mask all-ones (eval). torch-Linear convention: y = x @ W.T.

Sharding (head + output-row parallel, two AllToAlls, no all-reduce):
  - core c (0..7) owns heads {2c, 2c+1} for BOTH batches.
  - Phase A: per batch, project Q,K (transposed layout [d, s]) and V
    (natural layout [s, d]) for the core's 2 heads. bf16 inputs, fp32 acc.
  - Phase B: per (batch, q-block, head): scoresT tiles [k, q] via
    Kh^T-stationary matmuls, exp on ACT (1/sqrt(dk) folded into the
    activation scale), PV accumulation in the transposed layout (no
    attention transposes anywhere). Softmax denominators: DVE
    accumulation of exp tiles + one GPSIMD partition_all_reduce, so the
    PE never waits on softmax bookkeeping. Softmax without
    max-subtraction: scores are O(few) here, mathematically identical
    to the reference.
  - One AllToAll per batch (8 ranks, 1MB): fired as soon as that batch's
    attention finishes, so A2A#0 hides under batch-1 attention and A2A#1
    under batch-0's output projection.
  - Phase D: out_chunk = concat_chunk @ W_o.T per batch; each core
    produces 256 output rows per batch.
Host side: pre-transpose/cast inputs to bf16 (PE-tile-friendly layouts,
single big DMAs), slice weights per core, scatter-gather the per-core
[512, 2048] fp32 chunks into the full [B, S, HIDDEN] output.
"""

import math
from contextlib import ExitStack

import ml_dtypes
import numpy as np

import concourse.bass as bass
import concourse.bass_isa as bass_isa
import concourse.tile as tile
from concourse import bacc, mybir
from concourse.bass_utils import run_bass_kernel_spmd
from concourse.masks import make_identity

BF16 = mybir.dt.bfloat16
F32 = mybir.dt.float32
NPBF16 = ml_dtypes.bfloat16

HIDDEN = 2048
HEADS = 16
D_K = 128
B = 2
N_CORES = 8
HPC = HEADS // N_CORES          # heads per core (2)
DPC = HPC * D_K                 # concat cols per core (256)
NHT = HIDDEN // 128             # 16 hidden-dim 128-tiles


def _mha_kernel(ctx: ExitStack, tc: tile.TileContext, aps: dict, S: int):
    nc = tc.nc
    NKT = S // 128                   # seq 128-tiles
    SBLK = min(512, S)               # matmul moving-dim block
    NSB = S // SBLK
    QBLK = SBLK
    NQB = NSB
    SCB = S // N_CORES               # per-batch output rows per core (256)
    OBLK = 512
    NOB = HIDDEN // OBLK
    scale = 1.0 / math.sqrt(D_K)

    qT, kT, vT = aps["qT"], aps["kT"], aps["vT"]   # per batch [HIDDEN, S]
    wqT, wkT, wvT = aps["wqT"], aps["wkT"], aps["wvT"]  # [128, NHT*DPC]
    woT = aps["woT"]                                # [128, NHT*HIDDEN]
    out = aps["out"]                                # [B*SCB, HIDDEN] f32
    a2a_in = aps["a2a_in"]                          # per batch [8*DPC, SCB]
    a2a_out = aps["a2a_out"]                        # per batch [8*DPC, SCB]

    # ---- resident weights (pre-tiled on host; one big DMA each) ----
    w_pool = ctx.enter_context(tc.tile_pool(name="wqkv", bufs=1))
    wq_sb = w_pool.tile([128, NHT * DPC], BF16, tag="wq")
    wk_sb = w_pool.tile([128, NHT * DPC], BF16, tag="wk")
    wv_sb = w_pool.tile([128, NHT * DPC], BF16, tag="wv")
    nc.sync.dma_start(out=wq_sb[:], in_=wqT[:, :])

    # ---- identity for PE transposes ----
    id_pool = ctx.enter_context(tc.tile_pool(name="ident", bufs=1))
    identity = id_pool.tile([128, 128], BF16, tag="ident")
    make_identity(nc, identity[:])

    # ---- resident projection outputs (both batches) ----
    proj_pool = ctx.enter_context(tc.tile_pool(name="proj", bufs=1))
    qh_sb = [proj_pool.tile([128, HPC * S], BF16, tag=f"qh{b}", name=f"qh{b}")
             for b in range(B)]
    kh_sb = [proj_pool.tile([128, HPC * S], BF16, tag=f"kh{b}", name=f"kh{b}")
             for b in range(B)]
    vh_sb = [proj_pool.tile([128, NKT * DPC], BF16, tag=f"vh{b}", name=f"vh{b}")
             for b in range(B)]

    # ================= per-batch: A (proj) then B (attention) =========
    def phase_a(b):
        with tc.tile_pool(name="psA", bufs=8, space="PSUM") as psA, \
             tc.tile_pool(name="xrow", bufs=4) as xrow_pool:
            # qhT / khT: [d, s] layout.  psum[dt*NSB+sb] accumulates over ht.
            for wi, (src, w_sb, dst) in enumerate(
                    ((qT[b], wq_sb, qh_sb[b]), (kT[b], wk_sb, kh_sb[b]))):
                if b == 0 and wi == 0:
                    nc.sync.dma_start(out=wk_sb[:], in_=wkT[:, :])
                ps = [psA.tile([128, SBLK], F32, tag="psA", name=f"psA{i}")
                      for i in range(HPC * NSB)]
                for ht in range(NHT):
                    row = xrow_pool.tile([128, S], BF16, tag="xrow")
                    nc.sync.dma_start(out=row[:],
                                      in_=src[ht * 128:(ht + 1) * 128, :])
                    for dt in range(HPC):
                        for sb in range(NSB):
                            nc.tensor.matmul(
                                ps[dt * NSB + sb][:],
                                lhsT=w_sb[:, ht * DPC + dt * 128:
                                          ht * DPC + (dt + 1) * 128],
                                rhs=row[:, sb * SBLK:(sb + 1) * SBLK],
                                start=(ht == 0), stop=(ht == NHT - 1))
                for dt in range(HPC):
                    for sb in range(NSB):
                        nc.vector.tensor_copy(
                            dst[:, dt * S + sb * SBLK: dt * S + (sb + 1) * SBLK],
                            ps[dt * NSB + sb][:])

            # vh: project vhT like qhT (single vT pass, N=512 matmuls),
            # then PE-transpose [128,128] chunks into the natural layout.
            if b == 0:
                nc.sync.dma_start(out=wv_sb[:], in_=wvT[:, :])
            with tc.tile_pool(name="vfull", bufs=4) as vfull, \
                 tc.tile_pool(name="vtmp", bufs=1) as vtmp_pool:
                vhT_tmp = vtmp_pool.tile([128, HPC * S], BF16, tag="vtmp")
                ps = [psA.tile([128, SBLK], F32, tag="psA", name=f"psV{i}")
                      for i in range(HPC * NSB)]
                for ht in range(NHT):
                    vr = vfull.tile([128, S], BF16, tag="vrow")
                    nc.sync.dma_start(
                        out=vr[:], in_=vT[b][ht * 128:(ht + 1) * 128, :])
                    for dt in range(HPC):
                        for sb in range(NSB):
                            nc.tensor.matmul(
                                ps[dt * NSB + sb][:],
                                lhsT=wv_sb[:, ht * DPC + dt * 128:
                                           ht * DPC + (dt + 1) * 128],
                                rhs=vr[:, sb * SBLK:(sb + 1) * SBLK],
                                start=(ht == 0), stop=(ht == NHT - 1))
                for dt in range(HPC):
                    for sb in range(NSB):
                        nc.vector.tensor_copy(
                            vhT_tmp[:, dt * S + sb * SBLK:
                                    dt * S + (sb + 1) * SBLK],
                            ps[dt * NSB + sb][:])
                for st in range(NKT):
                    for dt in range(HPC):
                        pst = psA.tile([128, 128], BF16, tag="psA",
                                       name=f"psT{st}_{dt}")
                        nc.tensor.transpose(
                            pst[:],
                            vhT_tmp[:, dt * S + st * 128:
                                    dt * S + (st + 1) * 128],
                            identity[:])
                        nc.vector.tensor_copy(
                            vh_sb[b][:, st * DPC + dt * 128:
                                     st * DPC + (dt + 1) * 128],
                            pst[:])

    def phase_b(b):
        with tc.tile_pool(name="pss", bufs=5, space="PSUM") as pss_pool, \
             tc.tile_pool(name="pspv", bufs=3, space="PSUM") as pspv_pool, \
             tc.tile_pool(name="es", bufs=8) as es_pool, \
             tc.tile_pool(name="dtree", bufs=10) as dt_pool, \
             tc.tile_pool(name="acc", bufs=3) as acc_pool, \
             tc.tile_pool(name="rb", bufs=3) as rb_pool, \
             tc.tile_pool(name="ao", bufs=4) as ao_pool:
            for qb in range(NQB):
                for l in range(HPC):
                    pv = pspv_pool.tile([128, QBLK], F32, tag="pv")
                    # denominator tree: bf16 pair adds (2x DVE mode)
                    pending = {}
                    for kt in range(NKT):
                        lhs_k = kh_sb[b][:, l * S + kt * 128:
                                         l * S + (kt + 1) * 128]
                        lhs_v = vh_sb[b][:, kt * DPC + l * 128:
                                         kt * DPC + (l + 1) * 128]
                        ps = pss_pool.tile([128, QBLK], F32, tag="pss")
                        nc.tensor.matmul(
                            ps[:], lhsT=lhs_k,
                            rhs=qh_sb[b][:, l * S + qb * QBLK:
                                         l * S + (qb + 1) * QBLK],
                            start=True, stop=True)
                        es = es_pool.tile([128, QBLK], BF16, tag="es")
                        nc.scalar.activation(
                            es[:], ps[:], mybir.ActivationFunctionType.Exp,
                            scale=scale)
                        nc.tensor.matmul(
                            pv[:], lhsT=lhs_v, rhs=es[:],
                            start=(kt == 0), stop=(kt == NKT - 1))
                        # fold into the bf16 pair tree as tiles arrive
                        node, level = es, 0
                        while level in pending:
                            nxt = dt_pool.tile([128, QBLK], BF16, tag="dtree")
                            nc.vector.tensor_add(nxt[:], pending.pop(level)[:],
                                                 node[:])
                            node, level = nxt, level + 1
                        pending[level] = node
                    nodes = [pending[k] for k in sorted(pending)]
                    node = nodes[0]
                    for other in nodes[1:]:
                        nxt = dt_pool.tile([128, QBLK], BF16, tag="dtree")
                        nc.vector.tensor_add(nxt[:], node[:], other[:])
                        node = nxt
                    acc = acc_pool.tile([128, QBLK], F32, tag="acc")
                    nc.vector.tensor_copy(acc[:], node[:])
                    rb = rb_pool.tile([128, QBLK], F32, tag="rb")
                    nc.gpsimd.partition_all_reduce(
                        rb[:], acc[:], channels=128,
                        reduce_op=bass_isa.ReduceOp.add)
                    nc.vector.reciprocal_approx_fast(rb[:], rb[:])
                    ao = ao_pool.tile([128, QBLK], BF16, tag="ao")
                    nc.vector.tensor_mul(ao[:], pv[:], rb[:])
                    # scatter into a2a_in[b]: chunk m rows [m*DPC+l*128, +128)
                    q0 = qb * QBLK
                    while q0 < (qb + 1) * QBLK:
                        m = q0 // SCB
                        cend = min((qb + 1) * QBLK, (m + 1) * SCB)
                        nc.sync.dma_start(
                            out=a2a_in[b][m * DPC + l * 128:
                                          m * DPC + (l + 1) * 128,
                                          q0 - m * SCB: cend - m * SCB],
                            in_=ao[:, q0 - qb * QBLK: cend - qb * QBLK])
                        q0 = cend

    colls = []
    cc_tiles = []
    cc_pool = None
    wo_sb = None
    for b in range(B):
        phase_a(b)
        if b == 1:
            # W_o arrives during batch-1 attention (SBUF is free by then)
            wo_pool = ctx.enter_context(tc.tile_pool(name="wo", bufs=1))
            wo_sb = wo_pool.tile([128, NHT * HIDDEN], BF16, tag="wo")
            nc.sync.dma_start(out=wo_sb[:], in_=woT[:, :])
        phase_b(b)
        if cc_pool is None:
            cc_pool = ctx.enter_context(tc.tile_pool(name="cc", bufs=B))
        coll = nc.gpsimd.collective_compute(
            "AllToAll", mybir.AluOpType.bypass,
            replica_groups=[list(range(N_CORES))],
            ins=[a2a_in[b][:, :]], outs=[a2a_out[b][:, :]])
        colls.append(coll)
        cc_sb = cc_pool.tile([128, NHT * SCB], BF16, tag="cc", name=f"cc{b}")
        dma = nc.gpsimd.dma_start(
            out=cc_sb[:].rearrange("p (t s) -> p t s", t=NHT),
            in_=a2a_out[b][:, :].rearrange("(t p) s -> p t s", p=128))
        tile.add_dep_helper(dma.ins, coll.ins,
                            reason="a2a_out after collective")
        cc_tiles.append(cc_sb)

    # ================= Phase D: output projection (per batch) =========
    with tc.tile_pool(name="pso", bufs=3, space="PSUM") as pso_pool, \
         tc.tile_pool(name="osb", bufs=4) as osb_pool:
        NST = (SCB + 127) // 128
        for b in range(B):
            cc_sb = cc_tiles[b]
            for st in range(NST):
                mrows = min(128, SCB - st * 128)
                pso = [pso_pool.tile([128, OBLK], F32, tag="pso",
                                     name=f"pso{i}") for i in range(NOB)]
                for ht in range(NHT):
                    lhs = cc_sb[:, ht * SCB + st * 128:
                                ht * SCB + st * 128 + mrows]
                    for ot in range(NOB):
                        nc.tensor.matmul(
                            pso[ot][:mrows, :], lhsT=lhs,
                            rhs=wo_sb[:, ht * HIDDEN + ot * OBLK:
                                      ht * HIDDEN + (ot + 1) * OBLK],
                            start=(ht == 0), stop=(ht == NHT - 1))
                for ot in range(NOB):
                    osb = osb_pool.tile([128, OBLK], F32, tag="osb")
                    nc.vector.tensor_copy(osb[:mrows, :], pso[ot][:mrows, :])
                    nc.sync.dma_start(
                        out=out[b * SCB + st * 128: b * SCB + st * 128 + mrows,
                                ot * OBLK:(ot + 1) * OBLK],
                        in_=osb[:mrows, :])


def build_nc(S: int):
    nc = bacc.Bacc("TRN2", target_bir_lowering=False, debug=False,
                   enable_asserts=False, num_devices=N_CORES)
    SCB = S // N_CORES
    aps = {
        "qT": [nc.dram_tensor(f"qT{b}", [HIDDEN, S], BF16,
                              kind="ExternalInput").ap() for b in range(B)],
        "kT": [nc.dram_tensor(f"kT{b}", [HIDDEN, S], BF16,
                              kind="ExternalInput").ap() for b in range(B)],
        "vT": [nc.dram_tensor(f"vT{b}", [HIDDEN, S], BF16,
                              kind="ExternalInput").ap() for b in range(B)],
        "wqT": nc.dram_tensor("wqT", [128, NHT * DPC], BF16,
                              kind="ExternalInput").ap(),
        "wkT": nc.dram_tensor("wkT", [128, NHT * DPC], BF16,
                              kind="ExternalInput").ap(),
        "wvT": nc.dram_tensor("wvT", [128, NHT * DPC], BF16,
                              kind="ExternalInput").ap(),
        "woT": nc.dram_tensor("woT", [128, NHT * HIDDEN], BF16,
                              kind="ExternalInput").ap(),
        "out": nc.dram_tensor("out", [B * SCB, HIDDEN], F32,
                              kind="ExternalOutput").ap(),
        "a2a_in": [nc.dram_tensor(f"a2a_in{b}", [N_CORES * DPC, SCB],
                                  BF16).ap() for b in range(B)],
        "a2a_out": [nc.dram_tensor(f"a2a_out{b}", [N_CORES * DPC, SCB],
                                   BF16).ap() for b in range(B)],
    }
    with tile.TileContext(nc) as tc:
        with ExitStack() as ctx:
            _mha_kernel(ctx, tc, aps, S)
    nc.compile()
    return nc


_NC_CACHE: dict = {}


def _tile_weight(w_slice_T):
    """[H, D] -> [128, (H//128)*D] with 128-row tiles laid out consecutively."""
    H, D = w_slice_T.shape
    return np.ascontiguousarray(
        w_slice_T.reshape(H // 128, 128, D).transpose(1, 0, 2).reshape(
            128, (H // 128) * D))


def make_in_maps(q, k, v, w_q, w_k, w_v, w_o):
    """Host-side shard/cast. Returns per-core input dicts."""
    qT = [np.ascontiguousarray(q[b].T).astype(NPBF16) for b in range(B)]
    kT = [np.ascontiguousarray(k[b].T).astype(NPBF16) for b in range(B)]
    vT = [np.ascontiguousarray(v[b].T).astype(NPBF16) for b in range(B)]
    woT = _tile_weight(np.ascontiguousarray(w_o.T).astype(NPBF16))
    in_maps = []
    for c in range(N_CORES):
        d0 = c * DPC
        m = {}
        for b in range(B):
            m[f"qT{b}"] = qT[b]
            m[f"kT{b}"] = kT[b]
            m[f"vT{b}"] = vT[b]
        m["wqT"] = _tile_weight(
            np.ascontiguousarray(w_q[d0:d0 + DPC, :].T).astype(NPBF16))
        m["wkT"] = _tile_weight(
            np.ascontiguousarray(w_k[d0:d0 + DPC, :].T).astype(NPBF16))
        m["wvT"] = _tile_weight(
            np.ascontiguousarray(w_v[d0:d0 + DPC, :].T).astype(NPBF16))
        m["woT"] = woT
        in_maps.append(m)
    return in_maps


def kernel(q, k, v, mask, w_q, w_k, w_v, w_o, _trace=False):
    q = np.asarray(q, np.float32)
    k = np.asarray(k, np.float32)
    v = np.asarray(v, np.float32)
    mask = np.asarray(mask)
    w_q = np.asarray(w_q, np.float32)
    w_k = np.asarray(w_k, np.float32)
    w_v = np.asarray(w_v, np.float32)
    w_o = np.asarray(w_o, np.float32)
    S = q.shape[1]

    if not np.all(mask != 0):
        # General-mask fallback (never hit for the eval problem: mask is all
        # ones).  Computed on host for correctness.
        return _numpy_reference(q, k, v, mask, w_q, w_k, w_v, w_o)

    if S not in _NC_CACHE:
        _NC_CACHE[S] = build_nc(S)
    nc = _NC_CACHE[S]

    in_maps = make_in_maps(q, k, v, w_q, w_k, w_v, w_o)
    res = run_bass_kernel_spmd(nc, in_maps, core_ids=list(range(N_CORES)),
                               trace=_trace)

    SCB = S // N_CORES
    out = np.empty((B, S, HIDDEN), np.float32)
    for c in range(N_CORES):
        for b in range(B):
            out[b, c * SCB:(c + 1) * SCB, :] = \
                res.results[c]["out"][b * SCB:(b + 1) * SCB, :]
    if _trace:
        return out, res
    return out


def _numpy_reference(q, k, v, mask, w_q, w_k, w_v, w_o):
    Bn, S, H = q.shape
    dk = H // HEADS

    def split_heads(x, w):
        y = x @ w.T
        return y.reshape(Bn, S, HEADS, dk).transpose(0, 2, 1, 3)

    qh = split_heads(q, w_q)
    kh = split_heads(k, w_k)
    vh = split_heads(v, w_v)
    s = np.einsum("bhqd,bhkd->bhqk", qh, kh) / np.sqrt(np.float32(dk))
    s = np.where(mask[:, None, :, :] == 0, np.float32(-1e9), s)
    s = s - s.max(-1, keepdims=True)
    e = np.exp(s)
    a = e / e.sum(-1, keepdims=True)
    o = np.einsum("bhqk,bhkd->bhqd", a, vh)
    o = o.transpose(0, 2, 1, 3).reshape(Bn, S, H)
    return (o @ w_o.T).astype(np.float32)


# revision 21
# speedup vs baseline: 1.1302x; 1.0508x over previous
"""Multi-head attention forward on 8 Trainium2 NeuronCores (Bass/Tile).

Problem: B=2, S=2048, HIDDEN=2048, HEADS=16, D_K=128, fp32 I/O,
mask all-ones (eval). torch-Linear convention: y = x @ W.T.

Sharding (head + output-row parallel, two AllToAlls, no all-reduce):
  - core c (0..7) owns heads {2c, 2c+1} for BOTH batches.
  - Phase A: per batch, project Q,K (transposed layout [d, s]) and V
    (natural layout [s, d]) for the core's 2 heads. bf16 inputs, fp32 acc.
  - Phase B: per (batch, q-block, head): scoresT tiles [k, q] via
    Kh^T-stationary matmuls, exp on ACT (1/sqrt(dk) folded into the
    activation scale), PV accumulation in the transposed layout (no
    attention transposes anywhere). Softmax denominators: DVE
    accumulation of exp tiles + one GPSIMD partition_all_reduce, so the
    PE never waits on softmax bookkeeping. Softmax without
    max-subtraction: scores are O(few) here, mathematically identical
    to the reference.
  - One AllToAll per batch (8 ranks, 1MB): fired as soon as that batch's
    attention finishes, so A2A#0 hides under batch-1 attention and A2A#1
    under batch-0's output projection.
  - Phase D: out_chunk = concat_chunk @ W_o.T per batch; each core
    produces 256 output rows per batch.
Host side: pre-transpose/cast inputs to bf16 (PE-tile-friendly layouts,
single big DMAs), slice weights per core, scatter-gather the per-core
[512, 2048] fp32 chunks into the full [B, S, HIDDEN] output.
"""

import math
from contextlib import ExitStack

import ml_dtypes
import numpy as np

import concourse.bass as bass
import concourse.bass_isa as bass_isa
import concourse.tile as tile
from concourse import bacc, mybir
from concourse.bass_utils import run_bass_kernel_spmd
from concourse.masks import make_identity

BF16 = mybir.dt.bfloat16
F32 = mybir.dt.float32
NPBF16 = ml_dtypes.bfloat16

HIDDEN = 2048
HEADS = 16
D_K = 128
B = 2
N_CORES = 8
HPC = HEADS // N_CORES          # heads per core (2)
DPC = HPC * D_K                 # concat cols per core (256)
NHT = HIDDEN // 128             # 16 hidden-dim 128-tiles


def _mha_kernel(ctx: ExitStack, tc: tile.TileContext, aps: dict, S: int):
    nc = tc.nc
    NKT = S // 128                   # seq 128-tiles
    SBLK = min(512, S)               # matmul moving-dim block
    NSB = S // SBLK
    QBLK = SBLK
    NQB = NSB
    SCB = S // N_CORES               # per-batch output rows per core (256)
    OBLK = 512
    NOB = HIDDEN // OBLK
    scale = 1.0 / math.sqrt(D_K)

    qT, kT, vT = aps["qT"], aps["kT"], aps["vT"]   # per batch [HIDDEN, S]
    wqT, wkT, wvT = aps["wqT"], aps["wkT"], aps["wvT"]  # [128, NHT*DPC]
    woT = aps["woT"]                                # [128, NHT*HIDDEN]
    out = aps["out"]                                # [B*SCB, HIDDEN] f32
    a2a_in = aps["a2a_in"]                          # per batch [8*DPC, SCB]
    a2a_out = aps["a2a_out"]                        # per batch [8*DPC, SCB]

    # ---- resident weights (pre-tiled on host; one big DMA each) ----
    w_pool = ctx.enter_context(tc.tile_pool(name="wqkv", bufs=1))
    wq_sb = w_pool.tile([128, NHT * DPC], BF16, tag="wq")
    wk_sb = w_pool.tile([128, NHT * DPC], BF16, tag="wk")
    wv_sb = w_pool.tile([128, NHT * DPC], BF16, tag="wv")
    nc.sync.dma_start(out=wq_sb[:], in_=wqT[:, :])

    # ---- identity for PE transposes ----
    id_pool = ctx.enter_context(tc.tile_pool(name="ident", bufs=1))
    identity = id_pool.tile([128, 128], BF16, tag="ident")
    make_identity(nc, identity[:])

    # ---- resident projection outputs (both batches) ----
    proj_pool = ctx.enter_context(tc.tile_pool(name="proj", bufs=1))
    qh_sb = [proj_pool.tile([128, HPC * S], BF16, tag=f"qh{b}", name=f"qh{b}")
             for b in range(B)]
    kh_sb = [proj_pool.tile([128, HPC * S], BF16, tag=f"kh{b}", name=f"kh{b}")
             for b in range(B)]
    vh_sb = [proj_pool.tile([128, NKT * DPC], BF16, tag=f"vh{b}", name=f"vh{b}")
             for b in range(B)]

    # ================= per-batch: A (proj) then B (attention) =========
    def phase_a(b, psA):
        with tc.tile_pool(name="xrow", bufs=4) as xrow_pool:
            # qhT / khT: [d, s] layout.  psum[dt*NSB+sb] accumulates over ht.
            for wi, (src, w_sb, dst) in enumerate(
                    ((qT[b], wq_sb, qh_sb[b]), (kT[b], wk_sb, kh_sb[b]))):
                if b == 0 and wi == 0:
                    nc.sync.dma_start(out=wk_sb[:], in_=wkT[:, :])
                ps = [psA.tile([128, SBLK], F32, tag="psA", name=f"psA{i}")
                      for i in range(HPC * NSB)]
                for ht in range(NHT):
                    row = xrow_pool.tile([128, S], BF16, tag="xrow")
                    nc.sync.dma_start(out=row[:],
                                      in_=src[ht * 128:(ht + 1) * 128, :])
                    for dt in range(HPC):
                        for sb in range(NSB):
                            nc.tensor.matmul(
                                ps[dt * NSB + sb][:],
                                lhsT=w_sb[:, ht * DPC + dt * 128:
                                          ht * DPC + (dt + 1) * 128],
                                rhs=row[:, sb * SBLK:(sb + 1) * SBLK],
                                start=(ht == 0), stop=(ht == NHT - 1))
                for dt in range(HPC):
                    for sb in range(NSB):
                        nc.vector.tensor_copy(
                            dst[:, dt * S + sb * SBLK: dt * S + (sb + 1) * SBLK],
                            ps[dt * NSB + sb][:])

            # vh: project vhT like qhT (single vT pass, N=512 matmuls),
            # then PE-transpose [128,128] chunks into the natural layout.
            if b == 0:
                nc.sync.dma_start(out=wv_sb[:], in_=wvT[:, :])
            with tc.tile_pool(name="vfull", bufs=4) as vfull, \
                 tc.tile_pool(name="vtmp", bufs=1) as vtmp_pool:
                vhT_tmp = vtmp_pool.tile([128, HPC * S], BF16, tag="vtmp")
                ps = [psA.tile([128, SBLK], F32, tag="psA", name=f"psV{i}")
                      for i in range(HPC * NSB)]
                for ht in range(NHT):
                    vr = vfull.tile([128, S], BF16, tag="vrow")
                    nc.sync.dma_start(
                        out=vr[:], in_=vT[b][ht * 128:(ht + 1) * 128, :])
                    for dt in range(HPC):
                        for sb in range(NSB):
                            nc.tensor.matmul(
                                ps[dt * NSB + sb][:],
                                lhsT=wv_sb[:, ht * DPC + dt * 128:
                                           ht * DPC + (dt + 1) * 128],
                                rhs=vr[:, sb * SBLK:(sb + 1) * SBLK],
                                start=(ht == 0), stop=(ht == NHT - 1))
                for dt in range(HPC):
                    for sb in range(NSB):
                        nc.vector.tensor_copy(
                            vhT_tmp[:, dt * S + sb * SBLK:
                                    dt * S + (sb + 1) * SBLK],
                            ps[dt * NSB + sb][:])
                for st in range(NKT):
                    for dt in range(HPC):
                        pst = psA.tile([128, 128], BF16, tag="psA",
                                       name=f"psT{st}_{dt}")
                        nc.tensor.transpose(
                            pst[:],
                            vhT_tmp[:, dt * S + st * 128:
                                    dt * S + (st + 1) * 128],
                            identity[:])
                        nc.vector.tensor_copy(
                            vh_sb[b][:, st * DPC + dt * 128:
                                     st * DPC + (dt + 1) * 128],
                            pst[:])

    def phase_b(b):
        with tc.tile_pool(name="pss", bufs=5, space="PSUM") as pss_pool, \
             tc.tile_pool(name="pspv", bufs=3, space="PSUM") as pspv_pool, \
             tc.tile_pool(name="es", bufs=8) as es_pool, \
             tc.tile_pool(name="dtree", bufs=10) as dt_pool, \
             tc.tile_pool(name="acc", bufs=3) as acc_pool, \
             tc.tile_pool(name="rb", bufs=3) as rb_pool, \
             tc.tile_pool(name="ao", bufs=4) as ao_pool:
            for qb in range(NQB):
                for l in range(HPC):
                    pv = pspv_pool.tile([128, QBLK], F32, tag="pv")
                    # denominator tree: bf16 pair adds (2x DVE mode)
                    pending = {}
                    for kt in range(NKT):
                        lhs_k = kh_sb[b][:, l * S + kt * 128:
                                         l * S + (kt + 1) * 128]
                        lhs_v = vh_sb[b][:, kt * DPC + l * 128:
                                         kt * DPC + (l + 1) * 128]
                        ps = pss_pool.tile([128, QBLK], F32, tag="pss")
                        nc.tensor.matmul(
                            ps[:], lhsT=lhs_k,
                            rhs=qh_sb[b][:, l * S + qb * QBLK:
                                         l * S + (qb + 1) * QBLK],
                            start=True, stop=True)
                        es = es_pool.tile([128, QBLK], BF16, tag="es")
                        nc.scalar.activation(
                            es[:], ps[:], mybir.ActivationFunctionType.Exp,
                            scale=scale)
                        nc.tensor.matmul(
                            pv[:], lhsT=lhs_v, rhs=es[:],
                            start=(kt == 0), stop=(kt == NKT - 1))
                        # fold into the bf16 pair tree as tiles arrive
                        node, level = es, 0
                        while level in pending:
                            nxt = dt_pool.tile([128, QBLK], BF16, tag="dtree")
                            nc.vector.tensor_add(nxt[:], pending.pop(level)[:],
                                                 node[:])
                            node, level = nxt, level + 1
                        pending[level] = node
                    nodes = [pending[k] for k in sorted(pending)]
                    node = nodes[0]
                    for other in nodes[1:]:
                        nxt = dt_pool.tile([128, QBLK], BF16, tag="dtree")
                        nc.vector.tensor_add(nxt[:], node[:], other[:])
                        node = nxt
                    acc = acc_pool.tile([128, QBLK], F32, tag="acc")
                    nc.vector.tensor_copy(acc[:], node[:])
                    rb = rb_pool.tile([128, QBLK], F32, tag="rb")
                    nc.gpsimd.partition_all_reduce(
                        rb[:], acc[:], channels=128,
                        reduce_op=bass_isa.ReduceOp.add)
                    nc.vector.reciprocal_approx_fast(rb[:], rb[:])
                    ao = ao_pool.tile([128, QBLK], BF16, tag="ao")
                    nc.vector.tensor_mul(ao[:], pv[:], rb[:])
                    # scatter into a2a_in[b]: chunk m rows [m*DPC+l*128, +128)
                    q0 = qb * QBLK
                    while q0 < (qb + 1) * QBLK:
                        m = q0 // SCB
                        cend = min((qb + 1) * QBLK, (m + 1) * SCB)
                        nc.sync.dma_start(
                            out=a2a_in[b][m * DPC + l * 128:
                                          m * DPC + (l + 1) * 128,
                                          q0 - m * SCB: cend - m * SCB],
                            in_=ao[:, q0 - qb * QBLK: cend - qb * QBLK])
                        q0 = cend

    colls = []
    cc_tiles = []
    cc_pool = ctx.enter_context(tc.tile_pool(name="cc", bufs=B))
    with tc.tile_pool(name="psA", bufs=8, space="PSUM") as psA:
        for b in range(B):
            phase_a(b, psA)
    wo_pool = ctx.enter_context(tc.tile_pool(name="wo", bufs=1))
    wo_sb = wo_pool.tile([128, NHT * HIDDEN], BF16, tag="wo")
    nc.sync.dma_start(out=wo_sb[:], in_=woT[:, :])
    for b in range(B):
        phase_b(b)
        coll = nc.gpsimd.collective_compute(
            "AllToAll", mybir.AluOpType.bypass,
            replica_groups=[list(range(N_CORES))],
            ins=[a2a_in[b][:, :]], outs=[a2a_out[b][:, :]])
        colls.append(coll)
        cc_sb = cc_pool.tile([128, NHT * SCB], BF16, tag="cc", name=f"cc{b}")
        dma = nc.sync.dma_start(
            out=cc_sb[:].rearrange("p (t s) -> p t s", t=NHT),
            in_=a2a_out[b][:, :].rearrange("(t p) s -> p t s", p=128))
        tile.add_dep_helper(dma.ins, coll.ins,
                            reason="a2a_out after collective")
        cc_tiles.append(cc_sb)

    # ================= Phase D: output projection (per batch) =========
    with tc.tile_pool(name="pso", bufs=3, space="PSUM") as pso_pool, \
         tc.tile_pool(name="osb", bufs=4) as osb_pool:
        NST = (SCB + 127) // 128
        for b in range(B):
            cc_sb = cc_tiles[b]
            for st in range(NST):
                mrows = min(128, SCB - st * 128)
                pso = [pso_pool.tile([128, OBLK], F32, tag="pso",
                                     name=f"pso{i}") for i in range(NOB)]
                for ht in range(NHT):
                    lhs = cc_sb[:, ht * SCB + st * 128:
                                ht * SCB + st * 128 + mrows]
                    for ot in range(NOB):
                        nc.tensor.matmul(
                            pso[ot][:mrows, :], lhsT=lhs,
                            rhs=wo_sb[:, ht * HIDDEN + ot * OBLK:
                                      ht * HIDDEN + (ot + 1) * OBLK],
                            start=(ht == 0), stop=(ht == NHT - 1))
                for ot in range(NOB):
                    osb = osb_pool.tile([128, OBLK], F32, tag="osb")
                    nc.vector.tensor_copy(osb[:mrows, :], pso[ot][:mrows, :])
                    nc.gpsimd.dma_start(
                        out=out[b * SCB + st * 128: b * SCB + st * 128 + mrows,
                                ot * OBLK:(ot + 1) * OBLK],
                        in_=osb[:mrows, :])


def build_nc(S: int):
    nc = bacc.Bacc("TRN2", target_bir_lowering=False, debug=False,
                   enable_asserts=False, num_devices=N_CORES)
    SCB = S // N_CORES
    aps = {
        "qT": [nc.dram_tensor(f"qT{b}", [HIDDEN, S], BF16,
                              kind="ExternalInput").ap() for b in range(B)],
        "kT": [nc.dram_tensor(f"kT{b}", [HIDDEN, S], BF16,
                              kind="ExternalInput").ap() for b in range(B)],
        "vT": [nc.dram_tensor(f"vT{b}", [HIDDEN, S], BF16,
                              kind="ExternalInput").ap() for b in range(B)],
        "wqT": nc.dram_tensor("wqT", [128, NHT * DPC], BF16,
                              kind="ExternalInput").ap(),
        "wkT": nc.dram_tensor("wkT", [128, NHT * DPC], BF16,
                              kind="ExternalInput").ap(),
        "wvT": nc.dram_tensor("wvT", [128, NHT * DPC], BF16,
                              kind="ExternalInput").ap(),
        "woT": nc.dram_tensor("woT", [128, NHT * HIDDEN], BF16,
                              kind="ExternalInput").ap(),
        "out": nc.dram_tensor("out", [B * SCB, HIDDEN], F32,
                              kind="ExternalOutput").ap(),
        "a2a_in": [nc.dram_tensor(f"a2a_in{b}", [N_CORES * DPC, SCB],
                                  BF16).ap() for b in range(B)],
        "a2a_out": [nc.dram_tensor(f"a2a_out{b}", [N_CORES * DPC, SCB],
                                   BF16).ap() for b in range(B)],
    }
    with tile.TileContext(nc) as tc:
        with ExitStack() as ctx:
            _mha_kernel(ctx, tc, aps, S)
    nc.compile()
    return nc


_NC_CACHE: dict = {}


def _tile_weight(w_slice_T):
    """[H, D] -> [128, (H//128)*D] with 128-row tiles laid out consecutively."""
    H, D = w_slice_T.shape
    return np.ascontiguousarray(
        w_slice_T.reshape(H // 128, 128, D).transpose(1, 0, 2).reshape(
            128, (H // 128) * D))


def make_in_maps(q, k, v, w_q, w_k, w_v, w_o):
    """Host-side shard/cast. Returns per-core input dicts."""
    qT = [np.ascontiguousarray(q[b].T).astype(NPBF16) for b in range(B)]
    kT = [np.ascontiguousarray(k[b].T).astype(NPBF16) for b in range(B)]
    vT = [np.ascontiguousarray(v[b].T).astype(NPBF16) for b in range(B)]
    woT = _tile_weight(np.ascontiguousarray(w_o.T).astype(NPBF16))
    in_maps = []
    for c in range(N_CORES):
        d0 = c * DPC
        m = {}
        for b in range(B):
            m[f"qT{b}"] = qT[b]
            m[f"kT{b}"] = kT[b]
            m[f"vT{b}"] = vT[b]
        m["wqT"] = _tile_weight(
            np.ascontiguousarray(w_q[d0:d0 + DPC, :].T).astype(NPBF16))
        m["wkT"] = _tile_weight(
            np.ascontiguousarray(w_k[d0:d0 + DPC, :].T).astype(NPBF16))
        m["wvT"] = _tile_weight(
            np.ascontiguousarray(w_v[d0:d0 + DPC, :].T).astype(NPBF16))
        m["woT"] = woT
        in_maps.append(m)
    return in_maps


def kernel(q, k, v, mask, w_q, w_k, w_v, w_o, _trace=False):
    q = np.asarray(q, np.float32)
    k = np.asarray(k, np.float32)
    v = np.asarray(v, np.float32)
    mask = np.asarray(mask)
    w_q = np.asarray(w_q, np.float32)
    w_k = np.asarray(w_k, np.float32)
    w_v = np.asarray(w_v, np.float32)
    w_o = np.asarray(w_o, np.float32)
    S = q.shape[1]

    if not np.all(mask != 0):
        # General-mask fallback (never hit for the eval problem: mask is all
        # ones).  Computed on host for correctness.
        return _numpy_reference(q, k, v, mask, w_q, w_k, w_v, w_o)

    if S not in _NC_CACHE:
        _NC_CACHE[S] = build_nc(S)
    nc = _NC_CACHE[S]

    in_maps = make_in_maps(q, k, v, w_q, w_k, w_v, w_o)
    res = run_bass_kernel_spmd(nc, in_maps, core_ids=list(range(N_CORES)),
                               trace=_trace)

    SCB = S // N_CORES
    out = np.empty((B, S, HIDDEN), np.float32)
    for c in range(N_CORES):
        for b in range(B):
            out[b, c * SCB:(c + 1) * SCB, :] = \
                res.results[c]["out"][b * SCB:(b + 1) * SCB, :]
    if _trace:
        return out, res
    return out


def _numpy_reference(q, k, v, mask, w_q, w_k, w_v, w_o):
    Bn, S, H = q.shape
    dk = H // HEADS

    def split_heads(x, w):
        y = x @ w.T
        return y.reshape(Bn, S, HEADS, dk).transpose(0, 2, 1, 3)

    qh = split_heads(q, w_q)
    kh = split_heads(k, w_k)
    vh = split_heads(v, w_v)
    s = np.einsum("bhqd,bhkd->bhqk", qh, kh) / np.sqrt(np.float32(dk))
    s = np.where(mask[:, None, :, :] == 0, np.float32(-1e9), s)
    s = s - s.max(-1, keepdims=True)
    e = np.exp(s)
    a = e / e.sum(-1, keepdims=True)
    o = np.einsum("bhqk,bhkd->bhqd", a, vh)
    o = o.transpose(0, 2, 1, 3).reshape(Bn, S, H)
    return (o @ w_o.T).astype(np.float32)
